# revision 69
# baseline (speedup 1.0000x reference)
"""Point-transformer block kernel for TRN2 (8-core data-parallel).

Core i handles serialized patches 2i,2i+1: rows = order[2048i:2048(i+1)].

CPE uses the ~17% sparsity of the 3x3x3 neighbor taps: the host ships,
per core, the *valid* (point, offset) pairs only — pre-gathered neighbor
features in feature-major layout, k-grouped and padded to 128-pair
chunks. Stage 1 projects each chunk with its offset's weights (PE);
stage 2 accumulates the projected rows into h via dma_scatter_add
(race-free: within one k-group every destination row is distinct).
The center tap (k=13, always valid, identity-aligned) is computed
densely into SBUF. h = center + gather_transpose(scattered part).

Activations feature-major (FM): X^T [128 (c%128), CC (c//128), rows];
matmuls lhsT=W^T-arranged weights. bf16 matmuls, f32 residual.
"""
from contextlib import ExitStack

import numpy as np
import ml_dtypes

import concourse.bacc as bacc
import concourse.bass as bass
import concourse.mybir as mybir
import concourse.tile as tile

P = 128
C = 512
CC = C // P
NH = 8
HD = 64
KP = 1024
R = 2048
NPATCH = R // KP
NKK = 27
KCENTER = 13
NFULL = 16384
EPS = 1e-5
SCALE = (C // NH) ** -0.5
F32 = mybir.dt.float32
F32R = mybir.dt.float32r
BF16 = mybir.dt.bfloat16
F8 = mybir.dt.float8e4
I16 = mybir.dt.int16
AF = mybir.ActivationFunctionType
OP = mybir.AluOpType
DR = mybir.MatmulPerfMode.DoubleRow
WS = 16.0      # fp8 weight pre-scale (host multiplies weights by WS)
VD = 72        # v head width padded for DoubleRow (HD + denominator + pad)

HALF = 1024
NHALF = R // HALF
N512 = HALF // 512

HROWS = 17 * P          # h scratch rows: 2048 real + 128 trash/padding
TRASH = R               # scatter destination for padded pairs


def compute_nchunk(neighbor_idx, order):
    """Unified per-offset chunk counts (max over cores, 128-pair chunks)."""
    nbr = np.asarray(neighbor_idx)
    order = np.asarray(order)
    nchunk = {}
    for k in range(NKK):
        if k == KCENTER:
            continue
        mx = 0
        for c in range(8):
            rows = order[c * R:(c + 1) * R]
            mx = max(mx, int((nbr[rows, k] >= 0).sum()))
        if mx > 0:
            nchunk[k] = (mx + P - 1) // P
    return nchunk


def input_dram_specs(nchunk):
    """(name, shape, dtype) for every ExternalInput tensor."""
    ncpad = sum(nchunk.values()) * P
    specs = [
        ("featT_own", [P, CC, R], BF16),
        ("u_nc", [P, CC, ncpad], BF16),
        ("sca_idx", [P, ncpad // 16], I16),
        ("ident_idx", [P, R // 16], I16),
        ("wcat", [NKK, C, C], BF16),
        ("qkv_wT", [C, 3 * C], F8),
        ("proj_wT", [C, C], F8),
        ("fc1_wT", [C, 4 * C], F8),
        ("fc2_wT", [4 * C, C], F8),
    ]
    for nm, n in [("cpe_b", CC), ("cpe_ln_g", CC), ("cpe_ln_b", CC),
                  ("ln1_g", CC), ("ln1_b", CC), ("ln2_g", CC), ("ln2_b", CC),
                  ("q_b", CC), ("k_b", CC), ("proj_b", CC),
                  ("fc1_b", 4 * CC), ("fc2_b", CC)]:
        specs.append((nm, [P, n], F32))
    specs.append(("v_b_rep", [P, C], F32))
    return specs


def build_program(nchunk, gelu_exact=True, debug_taps=False):
    nc = bacc.Bacc("TRN2", target_bir_lowering=False, debug=False)

    dbg = {}

    def tap(name, ap):
        if not debug_taps:
            return
        t = nc.dram_tensor(f"dbg_{name}", list(ap.shape), ap.dtype,
                           kind="ExternalOutput")
        nc.sync.dma_start(t[:], ap)
        dbg[name] = t

    dram = {}
    for nm, shp, dt in input_dram_specs(nchunk):
        dram[nm] = nc.dram_tensor(nm, shp, dt, kind="ExternalInput")
    featT_own = dram["featT_own"]
    u_nc = dram["u_nc"]
    sca_idx = dram["sca_idx"]
    ident_idx = dram["ident_idx"]
    wcat = dram["wcat"]
    qkv_wT, proj_wT = dram["qkv_wT"], dram["proj_wT"]
    fc1_wT, fc2_wT = dram["fc1_wT"], dram["fc2_wT"]
    v_b_rep = dram["v_b_rep"]
    pvec_names = ["cpe_b", "cpe_ln_g", "cpe_ln_b", "ln1_g", "ln1_b",
                  "ln2_g", "ln2_b", "q_b", "k_b", "proj_b", "fc1_b", "fc2_b"]

    h_drams = [nc.dram_tensor(f"h_scratch{i}", [HROWS, C], BF16,
                              kind="Internal") for i in range(2)]
    outT = nc.dram_tensor("outT", [P, CC, R], F32, kind="ExternalOutput")

    with tile.TileContext(nc) as tc, ExitStack() as ctx:
        pers = ctx.enter_context(tc.tile_pool(name="pers", bufs=1))
        resid = ctx.enter_context(tc.tile_pool(name="resid", bufs=2))

        lnp = ctx.enter_context(tc.tile_pool(name="lnp", bufs=2))
        pv = {}
        for nm in pvec_names:
            t = pers.tile(list(dram[nm].shape), F32, tag=f"pv_{nm}")
            nc.sync.dma_start(t[:], dram[nm][:])
            pv[nm] = t
        v_b_t = pers.tile([P, C], F32, tag="v_b")
        nc.sync.dma_start(v_b_t[:], v_b_rep[:])
        ones_bf = pers.tile([P, P], BF16, tag="ones_bf")
        nc.vector.memset(ones_bf[:], 1.0)
        ones_f = pers.tile([P, P], F32, tag="ones_f")
        nc.vector.memset(ones_f[:], 1.0)
        ones1_bf = pers.tile([1, HD], BF16, tag="ones1_bf")
        nc.vector.memset(ones1_bf[:], 1.0)
        eps_t = pers.tile([P, 1], F32, tag="eps_t")
        nc.vector.memset(eps_t[:], EPS)

        def wload(pool, dram_ap, kdim, ndim, tag, dt=BF16):
            t = pool.tile([P, kdim // P, ndim], dt, tag=tag)
            nc.sync.dma_start(t[:], dram_ap.rearrange("(ko ki) n -> ki ko n", ki=P))
            return t

        def fm_ln_stats(lnp, x, x_is_f32):
            # returns (neg_m bf16, inv_std bf16, xa bf16) — the whole apply
            # chain then runs at the DVE's 2x 16-bit rate
            with tc.tile_pool(name="ln_ps", bufs=1, space="PSUM") as lps:
                sums_ps = lps.tile([P, R], F32, tag="ln_sums")
                sqs_ps = lps.tile([P, R], F32, tag="ln_sqs")
                if x_is_f32:
                    xa = lnp.tile([P, CC, R], BF16, tag="ln_xb")
                else:
                    xa = x
                for half in range(NHALF):
                    o = half * HALF
                    sq = lnp.tile([P, CC, HALF], BF16, tag="ln_sq")
                    nc.scalar.activation(sq[:], x[:, :, o:o + HALF], AF.Square)
                    if x_is_f32:
                        # bf16 shadow so the sums matmul runs at 1 cyc/row
                        nc.vector.tensor_copy(xa[:, :, o:o + HALF],
                                              x[:, :, o:o + HALF])
                    for kc in range(CC):
                        for nn in range(N512):
                            sl = slice(o + nn * 512, o + (nn + 1) * 512)
                            sli = slice(nn * 512, (nn + 1) * 512)
                            nc.tensor.matmul(sums_ps[:, sl], ones_bf[:],
                                             xa[:, kc, sl],
                                             start=(kc == 0), stop=(kc == CC - 1))
                            nc.tensor.matmul(sqs_ps[:, sl], ones_bf[:],
                                             sq[:, kc, sli],
                                             start=(kc == 0), stop=(kc == CC - 1))
                neg_m = lnp.tile([P, R], BF16, tag="ln_negm")
                nc.vector.tensor_scalar(neg_m[:], sums_ps[:], -1.0 / C, None,
                                        op0=OP.mult)
                msq = lnp.tile([P, R], F32, tag="ln_tmp")
                nc.scalar.activation(msq[:], neg_m[:], AF.Square)
                var = lnp.tile([P, R], F32, tag="ln_tmp2")
                nc.vector.scalar_tensor_tensor(var[:], sqs_ps[:], 1.0 / C, msq[:],
                                               op0=OP.mult, op1=OP.subtract)
            std = lnp.tile([P, R], F32, tag="ln_tmp")
            nc.scalar.activation(std[:], var[:], AF.Sqrt, bias=eps_t[:])
            inv_std = lnp.tile([P, R], F32, tag="ln_istd")
            nc.vector.reciprocal_approx_fast(inv_std[:], std[:])
            istd_bf = lnp.tile([P, R], BF16, tag="ln_istdb")
            nc.vector.tensor_copy(istd_bf[:], inv_std[:])
            return neg_m, istd_bf, xa

        def fm_ln_apply(lnp, xa, neg_m, inv_std, g, b, out, m, res=None,
                        cols=slice(0, R)):
            n = cols.stop - cols.start
            t_full = lnp.tile([P, R], BF16, tag="ln_t")
            t = t_full[:, :n]
            nc.vector.tensor_tensor(t, xa[:, m, cols], neg_m[:, cols], op=OP.add)
            nc.vector.tensor_tensor(t, t, inv_std[:, cols], op=OP.mult)
            if res is None:
                nc.vector.tensor_scalar(out[:, m, cols], t, g[:, m:m + 1],
                                        b[:, m:m + 1], op0=OP.mult, op1=OP.add)
            else:
                t2_full = lnp.tile([P, R], BF16, tag="ln_t2")
                t2 = t2_full[:, :n]
                nc.vector.tensor_scalar(t2, t, g[:, m:m + 1], b[:, m:m + 1],
                                        op0=OP.mult, op1=OP.add)
                nc.vector.tensor_tensor(out[:, m, cols], t2, res[:, m, cols],
                                        op=OP.add)

        def ln_stats_half(lnp, sums_ps, sqs_ps, x, cols, f32_in,
                          sq_on_act=True):
            """Per-half LN stats, reading x directly (f32 via f32r matmul).
            sums_ps/sqs_ps are [P, HALF] psum APs; they may alias (the sums
            group is fully consumed into neg_m before the sqs matmuls).
            Returns (neg_m bf16 [P,HALF], istd bf16 [P,HALF])."""
            n = cols.stop - cols.start
            sq = lnp.tile([P, CC, HALF], BF16, tag="ln_sq", name="ln_sq",
                          bufs=1)
            if sq_on_act:
                nc.scalar.activation(sq[:], x[:, :, cols], AF.Square)
            else:
                for kc in range(CC):
                    nc.vector.tensor_tensor(sq[:, kc, :], x[:, kc, cols],
                                            x[:, kc, cols], op=OP.mult)
            ones_l = ones_f[:].bitcast(F32R) if f32_in else ones_bf[:]
            for kc in range(CC):
                for nn in range(n // 512):
                    sl = slice(nn * 512, (nn + 1) * 512)
                    xsl = slice(cols.start + nn * 512, cols.start + (nn + 1) * 512)
                    rhs = x[:, kc, xsl]
                    if f32_in:
                        rhs = rhs.bitcast(F32R)
                    nc.tensor.matmul(sums_ps[:, sl], ones_l, rhs,
                                     start=(kc == 0), stop=(kc == CC - 1))
            neg_m = lnp.tile([P, HALF], BF16, tag="ln_negm", name="ln_negm")
            nc.vector.tensor_scalar(neg_m[:], sums_ps[:], -1.0 / C, None,
                                    op0=OP.mult)
            for kc in range(CC):
                for nn in range(n // 512):
                    sl = slice(nn * 512, (nn + 1) * 512)
                    nc.tensor.matmul(sqs_ps[:, sl], ones_bf[:], sq[:, kc, sl],
                                     start=(kc == 0), stop=(kc == CC - 1))
            msq = lnp.tile([P, HALF], F32, tag="ln_msq", name="ln_msq",
                           bufs=1)
            nc.scalar.activation(msq[:], neg_m[:], AF.Square)
            # var computed in place over msq, then std in place over var
            nc.vector.scalar_tensor_tensor(msq[:], sqs_ps[:], 1.0 / C, msq[:],
                                           op0=OP.mult, op1=OP.subtract)
            nc.scalar.activation(msq[:], msq[:], AF.Sqrt, bias=eps_t[:])
            istd_f = lnp.tile([P, HALF], F32, tag="ln_istdf", name="ln_istdf",
                              bufs=1)
            nc.vector.reciprocal_approx_fast(istd_f[:], msq[:])
            istd = lnp.tile([P, HALF], BF16, tag="ln_istd", name="ln_istd")
            nc.vector.tensor_copy(istd[:], istd_f[:])
            return neg_m, istd

        def ln_apply_half(lnp, x, neg_m, istd, g, b, out, m, cols, res=None):
            """out[:, m, cols] = LN-normalized x (+ res). x read directly."""
            t = lnp.tile([P, HALF], BF16, tag="ln_t", name="ln_t", bufs=1)
            nc.vector.tensor_tensor(t[:], x[:, m, cols], neg_m[:], op=OP.add)
            nc.vector.tensor_tensor(t[:], t[:], istd[:], op=OP.mult)
            if res is None:
                nc.vector.tensor_scalar(out[:, m, cols], t[:], g[:, m:m + 1],
                                        b[:, m:m + 1], op0=OP.mult, op1=OP.add)
            else:
                t2 = lnp.tile([P, HALF], BF16, tag="ln_t2", name="ln_t2",
                              bufs=1)
                nc.vector.tensor_scalar(t2[:], t[:], g[:, m:m + 1],
                                        b[:, m:m + 1], op0=OP.mult, op1=OP.add)
                nc.vector.tensor_tensor(out[:, m, cols], t2[:],
                                        res[:, m, cols], op=OP.add)

        feat1 = None

        from contextlib import nullcontext
        scope = getattr(nc, "named_scope", None) or (lambda _n: nullcontext())

        # =========================== CPE =================================
        # Phased: center + sparse(half0) -> gathers(0) -> sparse(half1),
        # then fin(0) [combine+LN -> feat1 h0] while half-1 scatters drain.
        # fin(1) is emitted from the back half, after qkv0, so the whole
        # half-0 chain and qkv0 overlap the half-1 sparse phase.
        front = ExitStack()
        hpool = front.enter_context(tc.tile_pool(name="hpool", bufs=1))
        fown = hpool.tile([P, CC, R], BF16, tag="fown")
        nc.sync.dma_start(fown[:], featT_own[:])
        h_base = hpool.tile([P, CC, R], BF16, tag="h_base")
        h2_sb = hpool.tile([P, CC, R], BF16, tag="h2")
        feat1 = resid.tile([P, CC, R], BF16, tag="resid")
        idt = hpool.tile([P, R // 16], I16, tag="idt")
        nc.sync.dma_start(idt[:], ident_idx[:])
        hrp = front.enter_context(tc.tile_pool(name="hrp", bufs=1))
        fps = front.enter_context(
            tc.tile_pool(name="fin_ps", bufs=1, space="PSUM"))

        def emit_gathers(half):
            o = half * HALF
            hrs = []
            for i, hd in enumerate(h_drams):
                hr = hrp.tile([P, CC, HALF], BF16, tag=f"h_rest{i}", name="hr")
                nc.gpsimd.dma_gather(hr[:], hd[:],
                                     idt[:, o // 16:(o + HALF) // 16],
                                     HALF, HALF, C, transpose=True,
                                     single_packet=False)
                hrs.append(hr)
            return hrs

        def emit_fin(half, hrs):
            o = half * HALF
            cols = slice(o, o + HALF)
            with scope(f"fin{half}"):
                for m in range(CC):
                    nc.vector.scalar_tensor_tensor(
                        h2_sb[:, m, cols], h_base[:, m, cols],
                        pv["cpe_b"][:, m:m + 1], hrs[0][:, m, :],
                        op0=OP.add, op1=OP.add)
                    nc.vector.tensor_tensor(
                        h2_sb[:, m, cols], h2_sb[:, m, cols],
                        hrs[1][:, m, :], op=OP.add)
                sums = fps.tile([P, HALF], F32, tag="lnc", name="lnc_ps")
                if half == 0:
                    # banks are free before the back half opens: separate
                    # sums/sqs tiles let the two stat groups overlap
                    with tc.tile_pool(name="lnc2_ps", bufs=1,
                                      space="PSUM") as xps:
                        sqs = xps.tile([P, HALF], F32, tag="lnc2",
                                       name="lnc2_ps")
                        neg_m, istd = ln_stats_half(lnp, sums, sqs, h2_sb,
                                                    cols, f32_in=False,
                                                    sq_on_act=False)
                else:
                    neg_m, istd = ln_stats_half(lnp, sums, sums, h2_sb, cols,
                                                f32_in=False, sq_on_act=False)
                for m in range(CC):
                    ln_apply_half(lnp, h2_sb, neg_m, istd,
                                  pv["cpe_ln_g"], pv["cpe_ln_b"],
                                  feat1, m, cols, res=fown)

        hrs0 = None
        with tc.tile_pool(name="hbp", bufs=1) as hbp:
            # ---- center tap: dense, identity-aligned, stays in SBUF ----
            with scope("cpe_center"), \
                 tc.tile_pool(name="w13p", bufs=1) as w13p, \
                 tc.tile_pool(name="cps", bufs=1, space="PSUM") as cps:
                w13 = wload(w13p, wcat[KCENTER], C, C, "w13")
                for m in range(CC):
                    for g in range(CC):
                        ps = cps.tile([P, 512], F32,
                                      tag=f"cps{(m * CC + g) % 4}")
                        for kc in range(CC):
                            nc.tensor.matmul(
                                ps[:], w13[:, kc, m * P:(m + 1) * P],
                                fown[:, kc, g * 512:(g + 1) * 512],
                                start=(kc == 0), stop=(kc == CC - 1))
                        nc.vector.tensor_copy(
                            h_base[:, m, g * 512:(g + 1) * 512], ps[:])

            # ---- sparse taps: project k-group chunks, scatter-add ----
            ncpad = sum(nchunk.values()) * P
            si = hbp.tile([P, ncpad // 16], I16, tag="si")
            nc.sync.dma_start(si[:], sca_idx[:])
            # small u-load pieces keep the DMA prefetch ahead of the PE
            items = list(nchunk.items())
            total = sum(nchunk.values())
            target = (total + 15) // 16
            groups, cur, acc = [], [], 0
            for k, nk in items:
                cur.append((k, nk))
                acc += nk
                if acc >= target:
                    groups.append(cur)
                    cur, acc = [], 0
            if cur:
                groups.append(cur)
            with (
                scope("cpe_sparse"),
                tc.tile_pool(name="upool", bufs=4) as up,
                tc.tile_pool(name="wstream", bufs=3) as wp,
                tc.tile_pool(name="zbp", bufs=7) as zbp,
                tc.tile_pool(name="cps2", bufs=1, space="PSUM") as cps2,
            ):
                off = 0
                gi = 0
                for grp in groups:
                    gsize = sum(nk for _, nk in grp)
                    u_t = up.tile([P, CC, gsize * P], BF16, tag="u_t")
                    nc.sync.dma_start(
                        u_t[:], u_nc[:, :, off:off + gsize * P])
                    if off == 0:
                        # zero the scatter accumulators; queued after the
                        # first compute loads, done before the first scatter
                        with scope("zinit"), \
                                tc.tile_pool(name="zinit", bufs=1) as zp:
                            zt = zp.tile([P, HROWS // P, C], BF16, tag="zt")
                            nc.vector.memset(zt[:], 0.0)
                            for hd in h_drams:
                                nc.sync.dma_start(
                                    hd[:].rearrange("(a ki) e -> ki a e",
                                                    ki=P), zt[:])
                    loff = 0
                    for k, nk in grp:
                        w_t = wload(wp, wcat[k], C, C, "w_t")
                        zb = zbp.tile([P, nk, C], BF16, tag="zb")
                        for j in range(nk):
                            ps = cps2.tile([P, C], F32, tag=f"ncps{j % 2}")
                            for kc in range(CC):
                                nc.tensor.matmul(
                                    ps[:], u_t[:, kc, loff + j * P:
                                               loff + (j + 1) * P],
                                    w_t[:, kc, :],
                                    start=(kc == 0), stop=(kc == CC - 1))
                            nc.scalar.activation(zb[:, j, :], ps[:],
                                                 AF.Identity)
                        nc.gpsimd.dma_scatter_add(
                            h_drams[gi % 2][:], zb[:],
                            si[:, off // 16:(off + nk * P) // 16],
                            nk * P, nk * P, C, single_packet=False)
                        off += nk * P
                        loff += nk * P
                        gi += 1
                hrs0 = emit_gathers(0)

        emit_fin(0, hrs0)

        # ============== back half: ln1+qkv+attn+proj+ln2+mlp =============
        # Per-patch pipeline: qkv half 1 and proj/ln2/mlp chunks are emitted
        # between attention heads so PE/DVE work fills the Act-bound (exp)
        # attention spans.
        QH = 512  # mlp row quarter
        gelu_f = AF.Gelu if gelu_exact else AF.Tanh
        with tc.tile_pool(name="attn", bufs=1, side="right") as ap_:
            q_sb = ap_.tile([P, CC, R], BF16, tag="q_sb")
            k_sb = ap_.tile([P, CC, R], BF16, tag="k_sb")
            v_sb = ap_.tile([P, R // P, NH, VD], F8, tag="v_sb")
            x1 = ap_.tile([P, CC, R], F8, tag="x1")
            feat2 = resid.tile([P, CC, R], BF16, tag="resid")
            nc.vector.memset(v_sb[:, :, :, HD:], 1.0 / WS)  # denom -> rec=WS/d

            # resident weights (fp8); the fc weights live in the post-front
            # mlp pool to keep the front/back SBUF peak under the cap
            w_q = wload(ap_, qkv_wT[:, 0:C], C, C, "w_q", dt=F8)
            w_k = wload(ap_, qkv_wT[:, C:2 * C], C, C, "w_k", dt=F8)
            w_v = wload(ap_, qkv_wT[:, 2 * C:3 * C], C, C, "w_v", dt=F8)
            w_pj = wload(ap_, proj_wT[:], C, C, "w_pj", dt=F8)

            bh = ExitStack()
            # PSUM is 8 banks x 2KB. Pre-attention: fps(2) + 3 wide tiles.
            # During attention: s_ps x2 + o_ps + one shared work tile.
            gem_tile = None  # set per phase

            ln1_sts = [None, None]

            def emit_ln1_stats(half):
                cols = slice(half * HALF, (half + 1) * HALF)
                sums = gem_tile(0)
                with scope(f"ln1s{half}"):
                    # sq on the DVE: half 1's Act Square would otherwise
                    # queue ahead of the first attention exp
                    ln1_sts[half] = ln_stats_half(lnp, sums, sums, feat1,
                                                  cols, f32_in=False,
                                                  sq_on_act=False)

            def emit_qkv(half, on_act=True):
                o = half * HALF
                cols = slice(o, o + HALF)
                neg_m, istd = ln1_sts[half]
                with scope(f"qkv{half}"):
                    for m in range(CC):
                        ln_apply_half(lnp, feat1, neg_m, istd, pv["ln1_g"],
                                      pv["ln1_b"], x1, m, cols)
                    # q_b is host-scaled by SCALE already
                    for part, (w_t, dst, bias, scl) in enumerate([
                            (w_q, q_sb, pv["q_b"], SCALE / WS),
                            (w_k, k_sb, pv["k_b"], 1.0 / WS)]):
                        for m in range(CC):
                            ps = gem_tile(part * CC + m)
                            for kc in range(0, CC, 2):
                                for nn in range(N512):
                                    sl = slice(nn * 512, (nn + 1) * 512)
                                    nc.tensor.matmul(
                                        ps[:, sl],
                                        w_t[:, kc:kc + 2, m * P:(m + 1) * P],
                                        x1[:, kc:kc + 2, o + nn * 512:
                                           o + (nn + 1) * 512],
                                        start=(kc == 0), stop=(kc == CC - 2),
                                        perf_mode=DR)
                            if on_act:
                                nc.scalar.activation(
                                    dst[:, m, cols], ps[:], AF.Identity,
                                    bias=bias[:, m:m + 1], scale=scl)
                            else:
                                # half 1 lands under the exp-saturated Act
                                nc.vector.tensor_scalar(
                                    dst[:, m, cols], ps[:], scl,
                                    bias[:, m:m + 1], op0=OP.mult, op1=OP.add)
                    for rc in range(HALF // P):
                        row0 = o + rc * P
                        vp = gem_tile(rc)[:, :C]
                        for kc in range(0, CC, 2):
                            nc.tensor.matmul(
                                vp, x1[:, kc:kc + 2, row0:row0 + P],
                                w_v[:, kc:kc + 2, :],
                                start=(kc == 0), stop=(kc == CC - 2),
                                perf_mode=DR)
                        nc.vector.scalar_tensor_tensor(
                            v_sb[:, row0 // P, :, :HD],
                            vp.rearrange("p (h d) -> p h d", d=HD),
                            1.0 / WS,
                            v_b_t[:].rearrange("p (h d) -> p h d", d=HD),
                            op0=OP.mult, op1=OP.add)

            with tc.tile_pool(name="pre_ps", bufs=1, space="PSUM") as pps0:
                def gem_tile(i):  # noqa: F811  (3 wide rotating tiles)
                    return pps0.tile([P, HALF], F32, tag=f"g{i % 3}",
                                     name="gem_ps")

                emit_ln1_stats(0)
                emit_qkv(0)
                # half-1 CPE finish lands here: its gathers queue behind the
                # half-1 scatters while qkv0 runs, then feat1 h1 completes
                emit_fin(1, emit_gathers(1))
                emit_ln1_stats(1)  # Act Square/Sqrt before exp table loads
            tap("h2", h2_sb[:])
            tap("feat1", feat1[:])
            front.close()

            msx = ExitStack()
            msb = msx.enter_context(tc.tile_pool(name="mlp_sb", bufs=1))
            o_sb = msb.tile([P, CC, R], F8, tag="o_sb")   # holds WS*o
            x2 = msb.tile([P, CC, R], F8, tag="x2")
            w_f1 = wload(msb, fc1_wT[:], C, 4 * C, "w_f1", dt=F8)
            w_f2 = wload(msb, fc2_wT[:], 4 * C, C, "w_f2", dt=F8)
            atx = ExitStack()
            ptp = atx.enter_context(tc.tile_pool(name="pT_pool", bufs=2))
            aps = atx.enter_context(
                tc.tile_pool(name="at_ps", bufs=1, space="PSUM"))
            aps2 = atx.enter_context(
                tc.tile_pool(name="at_ps2", bufs=2, space="PSUM"))
            wps = atx.enter_context(
                tc.tile_pool(name="work_ps", bufs=1, space="PSUM"))

            def gem_tile(i):  # noqa: F811  (single shared work tile)
                return wps.tile([P, HALF], F32, tag="w0", name="work_ps")

            def emit_proj(half, gtile=None):
                gtile = gtile or gem_tile
                o = half * HALF
                cols = slice(o, o + HALF)
                with scope(f"proj{half}"):
                    for m in range(CC):
                        ps = gtile(m)
                        for kc in range(0, CC, 2):
                            for nn in range(N512):
                                sl = slice(nn * 512, (nn + 1) * 512)
                                nc.tensor.matmul(
                                    ps[:, sl],
                                    w_pj[:, kc:kc + 2, m * P:(m + 1) * P],
                                    o_sb[:, kc:kc + 2, o + nn * 512:
                                         o + (nn + 1) * 512],
                                    start=(kc == 0), stop=(kc == CC - 2),
                                    perf_mode=DR)
                        tp_ = msb.tile([P, HALF], BF16, tag="pj_t",
                                       name="pj_t", bufs=2)
                        nc.vector.tensor_scalar(
                            tp_[:], ps[:], 1.0 / (WS * WS),
                            pv["proj_b"][:, m:m + 1], op0=OP.mult, op1=OP.add)
                        nc.vector.tensor_tensor(
                            feat2[:, m, cols], tp_[:], feat1[:, m, cols],
                            op=OP.add)

            def emit_ln2(half, gtile=None):
                gtile = gtile or gem_tile
                o = half * HALF
                cols = slice(o, o + HALF)
                with scope(f"ln2_{half}"):
                    sums = gtile(0)
                    neg_m, istd = ln_stats_half(lnp, sums, sums, feat2, cols,
                                                f32_in=False, sq_on_act=False)
                    for m in range(CC):
                        ln_apply_half(lnp, feat2, neg_m, istd, pv["ln2_g"],
                                      pv["ln2_b"], x2, m, cols)

            def emit_mlp_quarter(quarter, f1_tile, f2_tile):
                o = quarter * QH
                with scope(f"mlp{quarter}"):
                    g_sb = msb.tile([P, 4 * CC, QH], F8, tag="g_sb",
                                    name="g_sb")
                    for mm in range(4 * CC):
                        fp = f1_tile(mm)
                        for kc in range(0, CC, 2):
                            nc.tensor.matmul(
                                fp,
                                w_f1[:, kc:kc + 2, mm * P:(mm + 1) * P],
                                x2[:, kc:kc + 2, o:o + QH],
                                start=(kc == 0), stop=(kc == CC - 2),
                                perf_mode=DR)
                        nc.scalar.activation(g_sb[:, mm, :], fp, gelu_f,
                                             bias=pv["fc1_b"][:, mm:mm + 1],
                                             scale=1.0 / WS)
                    out_q = msb.tile([P, CC, QH], F32, tag="out_q",
                                     name="out_q")
                    for m in range(CC):
                        f2 = f2_tile(m)
                        for kc in range(0, 4 * CC, 2):
                            nc.tensor.matmul(
                                f2,
                                w_f2[:, kc:kc + 2, m * P:(m + 1) * P],
                                g_sb[:, kc:kc + 2, :],
                                start=(kc == 0), stop=(kc == 4 * CC - 2),
                                perf_mode=DR)
                        tq_ = msb.tile([P, QH], BF16, tag=f"tq{m % 2}",
                                       name="tq")
                        nc.vector.tensor_scalar(
                            tq_[:], f2, 1.0 / WS, pv["fc2_b"][:, m:m + 1],
                            op0=OP.mult, op1=OP.add)
                        nc.vector.tensor_tensor(
                            out_q[:, m, :], tq_[:], feat2[:, m, o:o + QH],
                            op=OP.add)
                    nc.sync.dma_start(outT[:, :, o:o + QH], out_q[:])

            def finish_head(st):
                # normalize tail of a previous head: deferred past the
                # next head's QK so the PE never waits on the DVE chain.
                # The reciprocal broadcast shares the work psum tile
                # (matmul outputs must sit at partition base 0); the o tail
                # is staged to SBUF first — the DVE can only read one PSUM
                # operand per instruction.
                ocp, rec_bf, hc, hpo, po = st
                rps = gem_tile(0)[:HD, :]
                for nn in range(KP // 512):
                    sl = slice(nn * 512, (nn + 1) * 512)
                    nc.tensor.matmul(rps[:, sl], ones1_bf[:],
                                     rec_bf[:, sl], start=True, stop=True)
                nc.vector.tensor_tensor(
                    o_sb[hpo:hpo + HD, hc, po:po + KP],
                    ocp[:], rps, op=OP.mult)

            # interleave schedule: after head index -> emitters.  MLP stays
            # out of the exp stream (gelu/exp share no act table: the
            # scheduler would alternate them, paying a table load each).
            chunks = {
                4: [lambda: emit_qkv(1, on_act=False)],
                9: [lambda: emit_proj(0)],
                11: [lambda: emit_ln2(0)],
            }

            with scope("attn"):
                pending = None
                hidx = 0
                for pt in range(NPATCH):
                    po = pt * KP
                    for h in range(NH):
                        hc, hpo = divmod(h * HD, P)
                        pT = ptp.tile([P, KP // P, KP], F8, tag="pT")
                        for jc in range(KP // P):
                            sps = aps2.tile([P, KP], F32, tag="s_ps")
                            for nn in range(KP // 512):
                                nc.tensor.matmul(
                                    sps[:, nn * 512:(nn + 1) * 512],
                                    k_sb[hpo:hpo + HD, hc,
                                         po + jc * P:po + (jc + 1) * P],
                                    q_sb[hpo:hpo + HD, hc,
                                         po + nn * 512:po + (nn + 1) * 512],
                                    start=True, stop=True)
                            nc.scalar.activation(pT[:, jc, :], sps[:], AF.Exp)
                        if pending is not None:
                            finish_head(pending)
                        ops_ = aps.tile([P, KP], F32, tag="o_ps")
                        for jc in range(0, KP // P, 2):
                            for nn in range(KP // 512):
                                sl = slice(nn * 512, (nn + 1) * 512)
                                nc.tensor.matmul(
                                    ops_[:VD, sl],
                                    v_sb[:, (po + jc * P) // P:
                                         (po + jc * P) // P + 2, h, :],
                                    pT[:, jc:jc + 2, sl],
                                    start=(jc == 0), stop=(jc == KP // P - 2),
                                    perf_mode=DR)
                        dcp = ptp.tile([1, KP], F32, tag="dcp", bufs=1)
                        nc.vector.tensor_copy(dcp[:], ops_[HD:HD + 1, :])
                        rec = ptp.tile([1, KP], F32, tag="rec", bufs=1)
                        nc.vector.reciprocal_approx_fast(rec[:], dcp[:])
                        rec_bf = ptp.tile([1, KP], BF16, tag="rec_bf")
                        nc.vector.tensor_copy(rec_bf[:], rec[:])
                        ocp = ptp.tile([HD, KP], BF16, tag="ocp")
                        nc.vector.tensor_copy(ocp[:], ops_[:HD, :])
                        pending = (ocp, rec_bf, hc, hpo, po)
                        for fn in chunks.get(hidx, []):
                            fn()
                        hidx += 1
                finish_head(pending)
                tap("o", o_sb[:])
                tap("q", q_sb[:])
                tap("k", k_sb[:])
                tap("v", v_sb[:])

            # tail: attention psum banks recycle into the mlp pools; proj1 +
            # ln2_1 go first (their Sqrt lands before the gelu run so the
            # act table only swaps twice), then all four quarters pipeline.
            atx.close()
            with tc.tile_pool(name="mlp_ps", bufs=1, space="PSUM") as mps:
                def f1_tile(i):
                    return mps.tile([P, QH], F32, tag=f"f1_{i % 2}",
                                    name="f1_ps")

                def f2_tile(i):
                    return mps.tile([P, QH], F32, tag=f"f2_{i % 2}",
                                    name="f2_ps")

                def gem_tile(i):  # noqa: F811  (tail wide tile: proj1/ln2_1)
                    return mps.tile([P, HALF], F32, tag="pj", name="pj_ps")

                emit_proj(1, gem_tile)
                emit_ln2(1, gem_tile)
                for quarter in range(R // QH):
                    emit_mlp_quarter(quarter, f1_tile, f2_tile)
            tap("feat2", feat2[:])
            tap("x2", x2[:])
            msx.close()
            bh.close()

    nc.compile()
    return nc


# ====================== host-side preparation ======================

def prep_shared(inputs):
    f32 = np.float32
    bf = ml_dtypes.bfloat16
    f8 = mybir.dt.np(F8)
    ws = np.float32(WS)

    def pp(v):
        return np.ascontiguousarray(np.asarray(v, f32).reshape(-1, P).T)

    def w8(v):  # fp8 weight, pre-scaled so values sit in e4m3's normal range
        return np.ascontiguousarray(np.asarray(v, f32).T * ws).astype(f8)

    qkv_b = np.asarray(inputs["qkv_b"], f32)
    ident = np.arange(R, dtype=np.int16).reshape(-1, 16).T  # [16, R//16]
    return dict(
        ident_idx=np.ascontiguousarray(np.tile(ident, (P // 16, 1))),
        wcat=np.ascontiguousarray(np.transpose(
            np.einsum("oc,kcd->kod", np.asarray(inputs["cpe_lin_w"], f32),
                      np.asarray(inputs["cpe_w"], f32)), (0, 2, 1))).astype(bf),
        qkv_wT=w8(inputs["qkv_w"]),
        proj_wT=w8(inputs["proj_w"]),
        fc1_wT=w8(inputs["fc1_w"]),
        fc2_wT=w8(inputs["fc2_w"]),
        cpe_b=pp(np.asarray(inputs["cpe_lin_w"], f32)
                 @ np.asarray(inputs["cpe_b"], f32)
                 + np.asarray(inputs["cpe_lin_b"], f32)),
        cpe_ln_g=pp(inputs["cpe_ln_g"]), cpe_ln_b=pp(inputs["cpe_ln_b"]),
        ln1_g=pp(inputs["ln1_g"]), ln1_b=pp(inputs["ln1_b"]),
        ln2_g=pp(inputs["ln2_g"]), ln2_b=pp(inputs["ln2_b"]),
        q_b=pp(qkv_b[:C] * SCALE), k_b=pp(qkv_b[C:2 * C]),
        v_b_rep=np.ascontiguousarray(np.broadcast_to(qkv_b[2 * C:], (P, C))),
        proj_b=pp(inputs["proj_b"]),
        fc1_b=pp(inputs["fc1_b"]), fc2_b=pp(inputs["fc2_b"]),
    )


def prep_core(inputs, core, nchunk):
    f32 = np.float32
    bf = ml_dtypes.bfloat16
    order = np.asarray(inputs["order"])
    feat = np.asarray(inputs["feat"], f32)
    nbr = np.asarray(inputs["neighbor_idx"])
    rows = order[core * R:(core + 1) * R]

    featT_own = np.ascontiguousarray(
        feat[rows].T.reshape(CC, P, R).transpose(1, 0, 2)).astype(bf)

    nb = nbr[rows]
    srcs, dsts = [], []
    for k, nk in nchunk.items():
        v = np.nonzero(nb[:, k] >= 0)[0]
        src = np.full(nk * P, NFULL, np.int64)
        dst = np.full(nk * P, TRASH, np.int64)
        src[:len(v)] = nb[v, k]
        dst[:len(v)] = v
        srcs.append(src)
        dsts.append(dst)
    src_all = np.concatenate(srcs)
    dst_all = np.concatenate(dsts)

    featp = np.vstack([feat, np.zeros((1, C), f32)])
    u = featp[src_all]                                    # [NCPAD, C]
    u_fm = np.ascontiguousarray(
        u.T.reshape(CC, P, -1).transpose(1, 0, 2)).astype(bf)

    sca = dst_all.astype(np.int16).reshape(-1, 16).T      # [16, NCPAD//16]
    sca_idx = np.ascontiguousarray(np.tile(sca, (P // 16, 1)))
    return dict(featT_own=featT_own, u_nc=u_fm, sca_idx=sca_idx), rows


def unshard_out(res_outT):
    return np.ascontiguousarray(
        np.asarray(res_outT).transpose(1, 0, 2).reshape(C, R).T)


# ======================= public entry point =======================

_CACHED = {}


def get_program(inputs):
    """Build (or fetch) the program for these inputs' sparsity pattern."""
    nchunk = compute_nchunk(inputs["neighbor_idx"], inputs["order"])
    key = tuple(sorted(nchunk.items()))
    if key not in _CACHED:
        _CACHED[key] = build_program(nchunk)
    return _CACHED[key], nchunk


def kernel(**inputs) -> np.ndarray:
    """Full-input, full-output entry. Shards across 8 NeuronCores by
    serialized patches (2 per core), runs the Bass kernel, scatters the
    per-core outputs back to original point order."""
    from concourse.bass_utils import run_bass_kernel_spmd

    inputs = {k: np.asarray(v) for k, v in inputs.items()}
    nc, nchunk = get_program(inputs)
    sh = prep_shared(inputs)
    in_maps, rows_l = [], []
    for c in range(8):
        ci, rows = prep_core(inputs, c, nchunk)
        in_maps.append({**sh, **ci})
        rows_l.append(rows)

    res = None
    last_err = None
    for attempt in range(3):
        try:
            res = run_bass_kernel_spmd(nc, in_maps, core_ids=list(range(8))).results
            break
        except Exception as e:   # transient NRT/axon hiccups: retry
            last_err = e
            import time as _t
            _t.sleep(2.0)
    if res is None:
        raise last_err

    out = np.zeros((NFULL, C), np.float32)
    for c in range(8):
        out[rows_l[c]] = unshard_out(res[c]["outT"])
    return out



# revision 71
# speedup vs baseline: 1.7677x; 1.7677x over previous
"""Point-transformer block kernel for TRN2 (8-core data-parallel).

Core i handles serialized patches 2i,2i+1: rows = order[2048i:2048(i+1)].

CPE uses the ~17% sparsity of the 3x3x3 neighbor taps: the host ships,
per core, the *valid* (point, offset) pairs only — pre-gathered neighbor
features in feature-major layout, k-grouped and padded to 128-pair
chunks. Stage 1 projects each chunk with its offset's weights (PE);
stage 2 accumulates the projected rows into h via dma_scatter_add
(race-free: within one k-group every destination row is distinct).
The center tap (k=13, always valid, identity-aligned) is computed
densely into SBUF. h = center + gather_transpose(scattered part).

Activations feature-major (FM): X^T [128 (c%128), CC (c//128), rows];
matmuls lhsT=W^T-arranged weights. bf16 matmuls, f32 residual.
"""
from contextlib import ExitStack

import numpy as np
import ml_dtypes

import concourse.bacc as bacc
import concourse.bass as bass
import concourse.mybir as mybir
import concourse.tile as tile

P = 128
C = 512
CC = C // P
NH = 8
HD = 64
KP = 1024
R = 2048
NPATCH = R // KP
NKK = 27
KCENTER = 13
NFULL = 16384
EPS = 1e-5
SCALE = (C // NH) ** -0.5
F32 = mybir.dt.float32
F32R = mybir.dt.float32r
BF16 = mybir.dt.bfloat16
F8 = mybir.dt.float8e4
I16 = mybir.dt.int16
AF = mybir.ActivationFunctionType
OP = mybir.AluOpType
DR = mybir.MatmulPerfMode.DoubleRow
WS = 16.0      # fp8 weight pre-scale (host multiplies weights by WS)
VD = 72        # v head width padded for DoubleRow (HD + denominator + pad)

HALF = 1024
NHALF = R // HALF
N512 = HALF // 512

HROWS = 17 * P          # h scratch rows: 2048 real + 128 trash/padding
TRASH = R               # scatter destination for padded pairs


def compute_nchunk(neighbor_idx, order):
    """Unified per-offset chunk counts (max over cores, 128-pair chunks)."""
    nbr = np.asarray(neighbor_idx)
    order = np.asarray(order)
    nchunk = {}
    for k in range(NKK):
        if k == KCENTER:
            continue
        mx = 0
        for c in range(8):
            rows = order[c * R:(c + 1) * R]
            mx = max(mx, int((nbr[rows, k] >= 0).sum()))
        if mx > 0:
            nchunk[k] = (mx + P - 1) // P
    return nchunk


def input_dram_specs(nchunk):
    """(name, shape, dtype) for every ExternalInput tensor."""
    ncpad = sum(nchunk.values()) * P
    specs = [
        ("featT_own", [P, CC, R], BF16),
        ("u_nc", [P, CC, ncpad], BF16),
        ("sca_idx", [P, ncpad // 16], I16),
        ("ident_idx", [P, R // 16], I16),
        ("wcat", [NKK, C, C], BF16),
        ("qkv_wT", [C, 3 * C], F8),
        ("proj_wT", [C, C], F8),
        ("fc1_wT", [C, 4 * C], F8),
        ("fc2_wT", [4 * C, C], F8),
    ]
    for nm, n in [("cpe_b", CC), ("cpe_ln_g", CC), ("cpe_ln_b", CC),
                  ("ln1_g", CC), ("ln1_b", CC), ("ln2_g", CC), ("ln2_b", CC),
                  ("q_b", CC), ("k_b", CC), ("proj_b", CC),
                  ("fc1_b", 4 * CC), ("fc2_b", CC)]:
        specs.append((nm, [P, n], F32))
    specs.append(("v_b_rep", [P, C], F32))
    return specs


def build_program(nchunk, gelu_exact=True, debug_taps=False):
    nc = bacc.Bacc("TRN2", target_bir_lowering=False, debug=False)

    dbg = {}

    def tap(name, ap):
        if not debug_taps:
            return
        t = nc.dram_tensor(f"dbg_{name}", list(ap.shape), ap.dtype,
                           kind="ExternalOutput")
        nc.sync.dma_start(t[:], ap)
        dbg[name] = t

    dram = {}
    for nm, shp, dt in input_dram_specs(nchunk):
        dram[nm] = nc.dram_tensor(nm, shp, dt, kind="ExternalInput")
    featT_own = dram["featT_own"]
    u_nc = dram["u_nc"]
    sca_idx = dram["sca_idx"]
    ident_idx = dram["ident_idx"]
    wcat = dram["wcat"]
    qkv_wT, proj_wT = dram["qkv_wT"], dram["proj_wT"]
    fc1_wT, fc2_wT = dram["fc1_wT"], dram["fc2_wT"]
    v_b_rep = dram["v_b_rep"]
    pvec_names = ["cpe_b", "cpe_ln_g", "cpe_ln_b", "ln1_g", "ln1_b",
                  "ln2_g", "ln2_b", "q_b", "k_b", "proj_b", "fc1_b", "fc2_b"]

    h_drams = [nc.dram_tensor(f"h_scratch{i}", [HROWS, C], BF16,
                              kind="Internal") for i in range(2)]
    outT = nc.dram_tensor("outT", [P, CC, R], F32, kind="ExternalOutput")

    with tile.TileContext(nc) as tc, ExitStack() as ctx:
        pers = ctx.enter_context(tc.tile_pool(name="pers", bufs=1))
        resid = ctx.enter_context(tc.tile_pool(name="resid", bufs=2))

        lnp = ctx.enter_context(tc.tile_pool(name="lnp", bufs=2))
        pv = {}
        for nm in pvec_names:
            t = pers.tile(list(dram[nm].shape), F32, tag=f"pv_{nm}")
            nc.sync.dma_start(t[:], dram[nm][:])
            pv[nm] = t
        v_b_t = pers.tile([P, C], F32, tag="v_b")
        nc.sync.dma_start(v_b_t[:], v_b_rep[:])
        ones_bf = pers.tile([P, P], BF16, tag="ones_bf")
        nc.vector.memset(ones_bf[:], 1.0)
        ones_f = pers.tile([P, P], F32, tag="ones_f")
        nc.vector.memset(ones_f[:], 1.0)
        ones1_bf = pers.tile([1, HD], BF16, tag="ones1_bf")
        nc.vector.memset(ones1_bf[:], 1.0)
        eps_t = pers.tile([P, 1], F32, tag="eps_t")
        nc.vector.memset(eps_t[:], EPS)

        def wload(pool, dram_ap, kdim, ndim, tag, dt=BF16):
            t = pool.tile([P, kdim // P, ndim], dt, tag=tag)
            nc.sync.dma_start(t[:], dram_ap.rearrange("(ko ki) n -> ki ko n", ki=P))
            return t

        def fm_ln_stats(lnp, x, x_is_f32):
            # returns (neg_m bf16, inv_std bf16, xa bf16) — the whole apply
            # chain then runs at the DVE's 2x 16-bit rate
            with tc.tile_pool(name="ln_ps", bufs=1, space="PSUM") as lps:
                sums_ps = lps.tile([P, R], F32, tag="ln_sums")
                sqs_ps = lps.tile([P, R], F32, tag="ln_sqs")
                if x_is_f32:
                    xa = lnp.tile([P, CC, R], BF16, tag="ln_xb")
                else:
                    xa = x
                for half in range(NHALF):
                    o = half * HALF
                    sq = lnp.tile([P, CC, HALF], BF16, tag="ln_sq")
                    nc.scalar.activation(sq[:], x[:, :, o:o + HALF], AF.Square)
                    if x_is_f32:
                        # bf16 shadow so the sums matmul runs at 1 cyc/row
                        nc.vector.tensor_copy(xa[:, :, o:o + HALF],
                                              x[:, :, o:o + HALF])
                    for kc in range(CC):
                        for nn in range(N512):
                            sl = slice(o + nn * 512, o + (nn + 1) * 512)
                            sli = slice(nn * 512, (nn + 1) * 512)
                            nc.tensor.matmul(sums_ps[:, sl], ones_bf[:],
                                             xa[:, kc, sl],
                                             start=(kc == 0), stop=(kc == CC - 1))
                            nc.tensor.matmul(sqs_ps[:, sl], ones_bf[:],
                                             sq[:, kc, sli],
                                             start=(kc == 0), stop=(kc == CC - 1))
                neg_m = lnp.tile([P, R], BF16, tag="ln_negm")
                nc.vector.tensor_scalar(neg_m[:], sums_ps[:], -1.0 / C, None,
                                        op0=OP.mult)
                msq = lnp.tile([P, R], F32, tag="ln_tmp")
                nc.scalar.activation(msq[:], neg_m[:], AF.Square)
                var = lnp.tile([P, R], F32, tag="ln_tmp2")
                nc.vector.scalar_tensor_tensor(var[:], sqs_ps[:], 1.0 / C, msq[:],
                                               op0=OP.mult, op1=OP.subtract)
            std = lnp.tile([P, R], F32, tag="ln_tmp")
            nc.scalar.activation(std[:], var[:], AF.Sqrt, bias=eps_t[:])
            inv_std = lnp.tile([P, R], F32, tag="ln_istd")
            nc.vector.reciprocal_approx_fast(inv_std[:], std[:])
            istd_bf = lnp.tile([P, R], BF16, tag="ln_istdb")
            nc.vector.tensor_copy(istd_bf[:], inv_std[:])
            return neg_m, istd_bf, xa

        def fm_ln_apply(lnp, xa, neg_m, inv_std, g, b, out, m, res=None,
                        cols=slice(0, R)):
            n = cols.stop - cols.start
            t_full = lnp.tile([P, R], BF16, tag="ln_t")
            t = t_full[:, :n]
            nc.vector.tensor_tensor(t, xa[:, m, cols], neg_m[:, cols], op=OP.add)
            nc.vector.tensor_tensor(t, t, inv_std[:, cols], op=OP.mult)
            if res is None:
                nc.vector.tensor_scalar(out[:, m, cols], t, g[:, m:m + 1],
                                        b[:, m:m + 1], op0=OP.mult, op1=OP.add)
            else:
                t2_full = lnp.tile([P, R], BF16, tag="ln_t2")
                t2 = t2_full[:, :n]
                nc.vector.tensor_scalar(t2, t, g[:, m:m + 1], b[:, m:m + 1],
                                        op0=OP.mult, op1=OP.add)
                nc.vector.tensor_tensor(out[:, m, cols], t2, res[:, m, cols],
                                        op=OP.add)

        def ln_stats_half(lnp, sums_ps, sqs_ps, x, cols, f32_in,
                          sq_on_act=True):
            """Per-half LN stats, reading x directly (f32 via f32r matmul).
            sums_ps/sqs_ps are [P, HALF] psum APs; they may alias (the sums
            group is fully consumed into neg_m before the sqs matmuls).
            Returns (neg_m bf16 [P,HALF], istd bf16 [P,HALF])."""
            n = cols.stop - cols.start
            sq = lnp.tile([P, CC, HALF], BF16, tag="ln_sq", name="ln_sq",
                          bufs=1)
            if sq_on_act:
                nc.scalar.activation(sq[:], x[:, :, cols], AF.Square)
            else:
                for kc in range(CC):
                    nc.vector.tensor_tensor(sq[:, kc, :], x[:, kc, cols],
                                            x[:, kc, cols], op=OP.mult)
            ones_l = ones_f[:].bitcast(F32R) if f32_in else ones_bf[:]
            for kc in range(CC):
                for nn in range(n // 512):
                    sl = slice(nn * 512, (nn + 1) * 512)
                    xsl = slice(cols.start + nn * 512, cols.start + (nn + 1) * 512)
                    rhs = x[:, kc, xsl]
                    if f32_in:
                        rhs = rhs.bitcast(F32R)
                    nc.tensor.matmul(sums_ps[:, sl], ones_l, rhs,
                                     start=(kc == 0), stop=(kc == CC - 1))
            neg_m = lnp.tile([P, HALF], BF16, tag="ln_negm", name="ln_negm")
            nc.vector.tensor_scalar(neg_m[:], sums_ps[:], -1.0 / C, None,
                                    op0=OP.mult)
            for kc in range(CC):
                for nn in range(n // 512):
                    sl = slice(nn * 512, (nn + 1) * 512)
                    nc.tensor.matmul(sqs_ps[:, sl], ones_bf[:], sq[:, kc, sl],
                                     start=(kc == 0), stop=(kc == CC - 1))
            msq = lnp.tile([P, HALF], F32, tag="ln_msq", name="ln_msq",
                           bufs=1)
            nc.scalar.activation(msq[:], neg_m[:], AF.Square)
            # var computed in place over msq, then std in place over var
            nc.vector.scalar_tensor_tensor(msq[:], sqs_ps[:], 1.0 / C, msq[:],
                                           op0=OP.mult, op1=OP.subtract)
            nc.scalar.activation(msq[:], msq[:], AF.Sqrt, bias=eps_t[:])
            istd_f = lnp.tile([P, HALF], F32, tag="ln_istdf", name="ln_istdf",
                              bufs=1)
            nc.vector.reciprocal_approx_fast(istd_f[:], msq[:])
            istd = lnp.tile([P, HALF], BF16, tag="ln_istd", name="ln_istd")
            nc.vector.tensor_copy(istd[:], istd_f[:])
            return neg_m, istd

        def ln_apply_half(lnp, x, neg_m, istd, g, b, out, m, cols, res=None):
            """out[:, m, cols] = LN-normalized x (+ res). x read directly."""
            t = lnp.tile([P, HALF], BF16, tag="ln_t", name="ln_t", bufs=1)
            nc.vector.tensor_tensor(t[:], x[:, m, cols], neg_m[:], op=OP.add)
            nc.vector.tensor_tensor(t[:], t[:], istd[:], op=OP.mult)
            if res is None:
                nc.vector.tensor_scalar(out[:, m, cols], t[:], g[:, m:m + 1],
                                        b[:, m:m + 1], op0=OP.mult, op1=OP.add)
            else:
                t2 = lnp.tile([P, HALF], BF16, tag="ln_t2", name="ln_t2",
                              bufs=1)
                nc.vector.tensor_scalar(t2[:], t[:], g[:, m:m + 1],
                                        b[:, m:m + 1], op0=OP.mult, op1=OP.add)
                nc.vector.tensor_tensor(out[:, m, cols], t2[:],
                                        res[:, m, cols], op=OP.add)

        feat1 = None

        from contextlib import nullcontext
        scope = getattr(nc, "named_scope", None) or (lambda _n: nullcontext())

        # =========================== CPE =================================
        # Phased: center + sparse(half0) -> gathers(0) -> sparse(half1),
        # then fin(0) [combine+LN -> feat1 h0] while half-1 scatters drain.
        # fin(1) is emitted from the back half, after qkv0, so the whole
        # half-0 chain and qkv0 overlap the half-1 sparse phase.
        front = ExitStack()
        hpool = front.enter_context(tc.tile_pool(name="hpool", bufs=1))
        fown = hpool.tile([P, CC, R], BF16, tag="fown")
        nc.sync.dma_start(fown[:], featT_own[:])
        h_base = hpool.tile([P, CC, R], BF16, tag="h_base")
        h2_sb = hpool.tile([P, CC, R], BF16, tag="h2")
        feat1 = resid.tile([P, CC, R], BF16, tag="resid")
        idt = hpool.tile([P, R // 16], I16, tag="idt")
        nc.sync.dma_start(idt[:], ident_idx[:])
        hrp = front.enter_context(tc.tile_pool(name="hrp", bufs=1))
        fps = front.enter_context(
            tc.tile_pool(name="fin_ps", bufs=1, space="PSUM"))

        def emit_gathers(half):
            o = half * HALF
            hrs = []
            for i, hd in enumerate(h_drams):
                hr = hrp.tile([P, CC, HALF], BF16, tag=f"h_rest{i}", name="hr")
                nc.gpsimd.dma_gather(hr[:], hd[:],
                                     idt[:, o // 16:(o + HALF) // 16],
                                     HALF, HALF, C, transpose=True,
                                     single_packet=False)
                hrs.append(hr)
            return hrs

        def emit_fin(half, hrs):
            o = half * HALF
            cols = slice(o, o + HALF)
            with scope(f"fin{half}"):
                for m in range(CC):
                    nc.vector.scalar_tensor_tensor(
                        h2_sb[:, m, cols], h_base[:, m, cols],
                        pv["cpe_b"][:, m:m + 1], hrs[0][:, m, :],
                        op0=OP.add, op1=OP.add)
                    nc.vector.tensor_tensor(
                        h2_sb[:, m, cols], h2_sb[:, m, cols],
                        hrs[1][:, m, :], op=OP.add)
                sums = fps.tile([P, HALF], F32, tag="lnc", name="lnc_ps")
                if half == 0:
                    # banks are free before the back half opens: separate
                    # sums/sqs tiles let the two stat groups overlap
                    with tc.tile_pool(name="lnc2_ps", bufs=1,
                                      space="PSUM") as xps:
                        sqs = xps.tile([P, HALF], F32, tag="lnc2",
                                       name="lnc2_ps")
                        neg_m, istd = ln_stats_half(lnp, sums, sqs, h2_sb,
                                                    cols, f32_in=False,
                                                    sq_on_act=False)
                else:
                    neg_m, istd = ln_stats_half(lnp, sums, sums, h2_sb, cols,
                                                f32_in=False, sq_on_act=False)
                for m in range(CC):
                    ln_apply_half(lnp, h2_sb, neg_m, istd,
                                  pv["cpe_ln_g"], pv["cpe_ln_b"],
                                  feat1, m, cols, res=fown)

        hrs0 = None
        with tc.tile_pool(name="hbp", bufs=1) as hbp:
            # ---- center tap: dense, identity-aligned, stays in SBUF ----
            with scope("cpe_center"), \
                 tc.tile_pool(name="w13p", bufs=1) as w13p, \
                 tc.tile_pool(name="cps", bufs=1, space="PSUM") as cps:
                w13 = wload(w13p, wcat[KCENTER], C, C, "w13")
                for m in range(CC):
                    for g in range(CC):
                        ps = cps.tile([P, 512], F32,
                                      tag=f"cps{(m * CC + g) % 4}")
                        for kc in range(CC):
                            nc.tensor.matmul(
                                ps[:], w13[:, kc, m * P:(m + 1) * P],
                                fown[:, kc, g * 512:(g + 1) * 512],
                                start=(kc == 0), stop=(kc == CC - 1))
                        nc.vector.tensor_copy(
                            h_base[:, m, g * 512:(g + 1) * 512], ps[:])

            # ---- sparse taps: project k-group chunks, scatter-add ----
            ncpad = sum(nchunk.values()) * P
            si = hbp.tile([P, ncpad // 16], I16, tag="si")
            nc.sync.dma_start(si[:], sca_idx[:])
            # small u-load pieces keep the DMA prefetch ahead of the PE
            items = list(nchunk.items())
            total = sum(nchunk.values())
            target = (total + 15) // 16
            groups, cur, acc = [], [], 0
            for k, nk in items:
                cur.append((k, nk))
                acc += nk
                if acc >= target:
                    groups.append(cur)
                    cur, acc = [], 0
            if cur:
                groups.append(cur)
            with (
                scope("cpe_sparse"),
                tc.tile_pool(name="upool", bufs=4) as up,
                tc.tile_pool(name="wstream", bufs=3) as wp,
                tc.tile_pool(name="zbp", bufs=7) as zbp,
                tc.tile_pool(name="cps2", bufs=1, space="PSUM") as cps2,
            ):
                off = 0
                gi = 0
                for grp in groups:
                    gsize = sum(nk for _, nk in grp)
                    u_t = up.tile([P, CC, gsize * P], BF16, tag="u_t")
                    nc.sync.dma_start(
                        u_t[:], u_nc[:, :, off:off + gsize * P])
                    if off == 0:
                        # zero the scatter accumulators; queued after the
                        # first compute loads, done before the first scatter
                        with scope("zinit"), \
                                tc.tile_pool(name="zinit", bufs=1) as zp:
                            zt = zp.tile([P, HROWS // P, C], BF16, tag="zt")
                            nc.vector.memset(zt[:], 0.0)
                            for hd in h_drams:
                                nc.sync.dma_start(
                                    hd[:].rearrange("(a ki) e -> ki a e",
                                                    ki=P), zt[:])
                    loff = 0
                    for k, nk in grp:
                        w_t = wload(wp, wcat[k], C, C, "w_t")
                        zb = zbp.tile([P, nk, C], BF16, tag="zb")
                        for j in range(nk):
                            ps = cps2.tile([P, C], F32, tag=f"ncps{j % 2}")
                            for kc in range(CC):
                                nc.tensor.matmul(
                                    ps[:], u_t[:, kc, loff + j * P:
                                               loff + (j + 1) * P],
                                    w_t[:, kc, :],
                                    start=(kc == 0), stop=(kc == CC - 1))
                            nc.scalar.activation(zb[:, j, :], ps[:],
                                                 AF.Identity)
                        nc.gpsimd.dma_scatter_add(
                            h_drams[gi % 2][:], zb[:],
                            si[:, off // 16:(off + nk * P) // 16],
                            nk * P, nk * P, C, single_packet=False)
                        off += nk * P
                        loff += nk * P
                        gi += 1
                hrs0 = emit_gathers(0)

        emit_fin(0, hrs0)

        # ============== back half: ln1+qkv+attn+proj+ln2+mlp =============
        # Per-patch pipeline: qkv half 1 and proj/ln2/mlp chunks are emitted
        # between attention heads so PE/DVE work fills the Act-bound (exp)
        # attention spans.
        QH = 512  # mlp row quarter
        gelu_f = AF.Gelu if gelu_exact else AF.Tanh
        with tc.tile_pool(name="attn", bufs=1, side="right") as ap_:
            q_sb = ap_.tile([P, CC, R], BF16, tag="q_sb")
            k_sb = ap_.tile([P, CC, R], BF16, tag="k_sb")
            v_sb = ap_.tile([P, R // P, NH, VD], F8, tag="v_sb")
            x1 = ap_.tile([P, CC, R], F8, tag="x1")
            feat2 = resid.tile([P, CC, R], BF16, tag="resid")
            nc.vector.memset(v_sb[:, :, :, HD:], 1.0 / WS)  # denom -> rec=WS/d

            # resident weights (fp8); the fc weights live in the post-front
            # mlp pool to keep the front/back SBUF peak under the cap
            w_q = wload(ap_, qkv_wT[:, 0:C], C, C, "w_q", dt=F8)
            w_k = wload(ap_, qkv_wT[:, C:2 * C], C, C, "w_k", dt=F8)
            w_v = wload(ap_, qkv_wT[:, 2 * C:3 * C], C, C, "w_v", dt=F8)
            w_pj = wload(ap_, proj_wT[:], C, C, "w_pj", dt=F8)

            bh = ExitStack()
            # PSUM is 8 banks x 2KB. Pre-attention: fps(2) + 3 wide tiles.
            # During attention: s_ps x2 + o_ps + one shared work tile.
            gem_tile = None  # set per phase

            ln1_sts = [None, None]

            def emit_ln1_stats(half):
                cols = slice(half * HALF, (half + 1) * HALF)
                sums = gem_tile(0)
                with scope(f"ln1s{half}"):
                    # sq on the DVE: half 1's Act Square would otherwise
                    # queue ahead of the first attention exp
                    ln1_sts[half] = ln_stats_half(lnp, sums, sums, feat1,
                                                  cols, f32_in=False,
                                                  sq_on_act=False)

            def emit_qkv(half, on_act=True):
                o = half * HALF
                cols = slice(o, o + HALF)
                neg_m, istd = ln1_sts[half]
                with scope(f"qkv{half}"):
                    for m in range(CC):
                        ln_apply_half(lnp, feat1, neg_m, istd, pv["ln1_g"],
                                      pv["ln1_b"], x1, m, cols)
                    # q_b is host-scaled by SCALE already
                    for part, (w_t, dst, bias, scl) in enumerate([
                            (w_q, q_sb, pv["q_b"], SCALE / WS),
                            (w_k, k_sb, pv["k_b"], 1.0 / WS)]):
                        for m in range(CC):
                            ps = gem_tile(part * CC + m)
                            for kc in range(0, CC, 2):
                                for nn in range(N512):
                                    sl = slice(nn * 512, (nn + 1) * 512)
                                    nc.tensor.matmul(
                                        ps[:, sl],
                                        w_t[:, kc:kc + 2, m * P:(m + 1) * P],
                                        x1[:, kc:kc + 2, o + nn * 512:
                                           o + (nn + 1) * 512],
                                        start=(kc == 0), stop=(kc == CC - 2),
                                        perf_mode=DR)
                            if on_act:
                                nc.scalar.activation(
                                    dst[:, m, cols], ps[:], AF.Identity,
                                    bias=bias[:, m:m + 1], scale=scl)
                            else:
                                # half 1 lands under the exp-saturated Act
                                nc.vector.tensor_scalar(
                                    dst[:, m, cols], ps[:], scl,
                                    bias[:, m:m + 1], op0=OP.mult, op1=OP.add)
                    for rc in range(HALF // P):
                        row0 = o + rc * P
                        vp = gem_tile(rc)[:, :C]
                        for kc in range(0, CC, 2):
                            nc.tensor.matmul(
                                vp, x1[:, kc:kc + 2, row0:row0 + P],
                                w_v[:, kc:kc + 2, :],
                                start=(kc == 0), stop=(kc == CC - 2),
                                perf_mode=DR)
                        nc.vector.scalar_tensor_tensor(
                            v_sb[:, row0 // P, :, :HD],
                            vp.rearrange("p (h d) -> p h d", d=HD),
                            1.0 / WS,
                            v_b_t[:].rearrange("p (h d) -> p h d", d=HD),
                            op0=OP.mult, op1=OP.add)

            with tc.tile_pool(name="pre_ps", bufs=1, space="PSUM") as pps0:
                def gem_tile(i):  # noqa: F811  (3 wide rotating tiles)
                    return pps0.tile([P, HALF], F32, tag=f"g{i % 3}",
                                     name="gem_ps")

                emit_ln1_stats(0)
                emit_qkv(0)
                # half-1 CPE finish lands here: its gathers queue behind the
                # half-1 scatters while qkv0 runs, then feat1 h1 completes
                emit_fin(1, emit_gathers(1))
                emit_ln1_stats(1)  # Act Square/Sqrt before exp table loads
            tap("h2", h2_sb[:])
            tap("feat1", feat1[:])
            front.close()

            msx = ExitStack()
            msb = msx.enter_context(tc.tile_pool(name="mlp_sb", bufs=1))
            o_sb = msb.tile([P, CC, R], F8, tag="o_sb")   # holds WS*o
            x2 = msb.tile([P, CC, R], F8, tag="x2")
            w_f1 = wload(msb, fc1_wT[:], C, 4 * C, "w_f1", dt=F8)
            w_f2 = wload(msb, fc2_wT[:], 4 * C, C, "w_f2", dt=F8)
            atx = ExitStack()
            ptp = atx.enter_context(tc.tile_pool(name="pT_pool", bufs=2))
            aps = atx.enter_context(
                tc.tile_pool(name="at_ps", bufs=1, space="PSUM"))
            aps2 = atx.enter_context(
                tc.tile_pool(name="at_ps2", bufs=2, space="PSUM"))
            wps = atx.enter_context(
                tc.tile_pool(name="work_ps", bufs=1, space="PSUM"))

            def gem_tile(i):  # noqa: F811  (single shared work tile)
                return wps.tile([P, HALF], F32, tag="w0", name="work_ps")

            def emit_proj(half, gtile=None):
                gtile = gtile or gem_tile
                o = half * HALF
                cols = slice(o, o + HALF)
                with scope(f"proj{half}"):
                    for m in range(CC):
                        ps = gtile(m)
                        for kc in range(0, CC, 2):
                            for nn in range(N512):
                                sl = slice(nn * 512, (nn + 1) * 512)
                                nc.tensor.matmul(
                                    ps[:, sl],
                                    w_pj[:, kc:kc + 2, m * P:(m + 1) * P],
                                    o_sb[:, kc:kc + 2, o + nn * 512:
                                         o + (nn + 1) * 512],
                                    start=(kc == 0), stop=(kc == CC - 2),
                                    perf_mode=DR)
                        tp_ = msb.tile([P, HALF], BF16, tag="pj_t",
                                       name="pj_t", bufs=2)
                        nc.vector.tensor_scalar(
                            tp_[:], ps[:], 1.0 / (WS * WS),
                            pv["proj_b"][:, m:m + 1], op0=OP.mult, op1=OP.add)
                        nc.vector.tensor_tensor(
                            feat2[:, m, cols], tp_[:], feat1[:, m, cols],
                            op=OP.add)

            def emit_ln2(half, gtile=None):
                gtile = gtile or gem_tile
                o = half * HALF
                cols = slice(o, o + HALF)
                with scope(f"ln2_{half}"):
                    sums = gtile(0)
                    neg_m, istd = ln_stats_half(lnp, sums, sums, feat2, cols,
                                                f32_in=False, sq_on_act=False)
                    for m in range(CC):
                        ln_apply_half(lnp, feat2, neg_m, istd, pv["ln2_g"],
                                      pv["ln2_b"], x2, m, cols)

            def emit_mlp_quarter(quarter, f1_tile, f2_tile):
                o = quarter * QH
                with scope(f"mlp{quarter}"):
                    g_sb = msb.tile([P, 4 * CC, QH], F8, tag="g_sb",
                                    name="g_sb")
                    for mm in range(4 * CC):
                        fp = f1_tile(mm)
                        for kc in range(0, CC, 2):
                            nc.tensor.matmul(
                                fp,
                                w_f1[:, kc:kc + 2, mm * P:(mm + 1) * P],
                                x2[:, kc:kc + 2, o:o + QH],
                                start=(kc == 0), stop=(kc == CC - 2),
                                perf_mode=DR)
                        nc.scalar.activation(g_sb[:, mm, :], fp, gelu_f,
                                             bias=pv["fc1_b"][:, mm:mm + 1],
                                             scale=1.0 / WS)
                    out_q = msb.tile([P, CC, QH], F32, tag="out_q",
                                     name="out_q")
                    for m in range(CC):
                        f2 = f2_tile(m)
                        for kc in range(0, 4 * CC, 2):
                            nc.tensor.matmul(
                                f2,
                                w_f2[:, kc:kc + 2, m * P:(m + 1) * P],
                                g_sb[:, kc:kc + 2, :],
                                start=(kc == 0), stop=(kc == 4 * CC - 2),
                                perf_mode=DR)
                        tq_ = msb.tile([P, QH], BF16, tag=f"tq{m % 2}",
                                       name="tq")
                        nc.vector.tensor_scalar(
                            tq_[:], f2, 1.0 / WS, pv["fc2_b"][:, m:m + 1],
                            op0=OP.mult, op1=OP.add)
                        nc.vector.tensor_tensor(
                            out_q[:, m, :], tq_[:], feat2[:, m, o:o + QH],
                            op=OP.add)
                    nc.sync.dma_start(outT[:, :, o:o + QH], out_q[:])

            def finish_head(st):
                # normalize tail of a previous head: deferred past the
                # next head's QK so the PE never waits on the DVE chain.
                # The reciprocal broadcast shares the work psum tile
                # (matmul outputs must sit at partition base 0); the o tail
                # is staged to SBUF first — the DVE can only read one PSUM
                # operand per instruction.
                ocp, rec_bf, hc, hpo, po = st
                rps = gem_tile(0)[:HD, :]
                for nn in range(KP // 512):
                    sl = slice(nn * 512, (nn + 1) * 512)
                    nc.tensor.matmul(rps[:, sl], ones1_bf[:],
                                     rec_bf[:, sl], start=True, stop=True)
                nc.vector.tensor_tensor(
                    o_sb[hpo:hpo + HD, hc, po:po + KP],
                    ocp[:], rps, op=OP.mult)

            # interleave schedule: after head index -> emitters.  MLP stays
            # out of the exp stream (gelu/exp share no act table: the
            # scheduler would alternate them, paying a table load each).
            chunks = {
                4: [lambda: emit_qkv(1, on_act=False)],
                9: [lambda: emit_proj(0)],
                11: [lambda: emit_ln2(0)],
            }

            with scope("attn"):
                pending = None
                hidx = 0
                for pt in range(NPATCH):
                    po = pt * KP
                    for h in range(NH):
                        hc, hpo = divmod(h * HD, P)
                        pT = ptp.tile([P, KP // P, KP], F8, tag="pT")
                        for jc in range(KP // P):
                            sps = aps2.tile([P, KP], F32, tag="s_ps")
                            for nn in range(KP // 512):
                                nc.tensor.matmul(
                                    sps[:, nn * 512:(nn + 1) * 512],
                                    k_sb[hpo:hpo + HD, hc,
                                         po + jc * P:po + (jc + 1) * P],
                                    q_sb[hpo:hpo + HD, hc,
                                         po + nn * 512:po + (nn + 1) * 512],
                                    start=True, stop=True)
                            nc.scalar.activation(pT[:, jc, :], sps[:], AF.Exp)
                        if pending is not None:
                            finish_head(pending)
                        ops_ = aps.tile([P, KP], F32, tag="o_ps")
                        for jc in range(0, KP // P, 2):
                            for nn in range(KP // 512):
                                sl = slice(nn * 512, (nn + 1) * 512)
                                nc.tensor.matmul(
                                    ops_[:VD, sl],
                                    v_sb[:, (po + jc * P) // P:
                                         (po + jc * P) // P + 2, h, :],
                                    pT[:, jc:jc + 2, sl],
                                    start=(jc == 0), stop=(jc == KP // P - 2),
                                    perf_mode=DR)
                        dcp = ptp.tile([1, KP], F32, tag="dcp", bufs=1)
                        nc.vector.tensor_copy(dcp[:], ops_[HD:HD + 1, :])
                        rec = ptp.tile([1, KP], F32, tag="rec", bufs=1)
                        nc.vector.reciprocal_approx_fast(rec[:], dcp[:])
                        rec_bf = ptp.tile([1, KP], BF16, tag="rec_bf")
                        nc.vector.tensor_copy(rec_bf[:], rec[:])
                        ocp = ptp.tile([HD, KP], BF16, tag="ocp")
                        nc.vector.tensor_copy(ocp[:], ops_[:HD, :])
                        pending = (ocp, rec_bf, hc, hpo, po)
                        for fn in chunks.get(hidx, []):
                            fn()
                        hidx += 1
                finish_head(pending)
                tap("o", o_sb[:])
                tap("q", q_sb[:])
                tap("k", k_sb[:])
                tap("v", v_sb[:])

            # tail: attention psum banks recycle into the mlp pools; proj1 +
            # ln2_1 go first (their Sqrt lands before the gelu run so the
            # act table only swaps twice), then all four quarters pipeline.
            atx.close()
            with tc.tile_pool(name="mlp_ps", bufs=1, space="PSUM") as mps:
                def f1_tile(i):
                    return mps.tile([P, QH], F32, tag=f"f1_{i % 2}",
                                    name="f1_ps")

                def f2_tile(i):
                    return mps.tile([P, QH], F32, tag=f"f2_{i % 2}",
                                    name="f2_ps")

                def gem_tile(i):  # noqa: F811  (tail wide tile: proj1/ln2_1)
                    return mps.tile([P, HALF], F32, tag="pj", name="pj_ps")

                emit_proj(1, gem_tile)
                emit_ln2(1, gem_tile)
                for quarter in range(R // QH):
                    emit_mlp_quarter(quarter, f1_tile, f2_tile)
            tap("feat2", feat2[:])
            tap("x2", x2[:])
            msx.close()
            bh.close()

    nc.compile()
    return nc


# ====================== host-side preparation ======================

def prep_shared(inputs):
    f32 = np.float32
    bf = ml_dtypes.bfloat16
    f8 = mybir.dt.np(F8)
    ws = np.float32(WS)

    def pp(v):
        return np.ascontiguousarray(np.asarray(v, f32).reshape(-1, P).T)

    def w8(v):  # fp8 weight, pre-scaled so values sit in e4m3's normal range
        return np.ascontiguousarray(np.asarray(v, f32).T * ws).astype(f8)

    qkv_b = np.asarray(inputs["qkv_b"], f32)
    ident = np.arange(R, dtype=np.int16).reshape(-1, 16).T  # [16, R//16]
    return dict(
        ident_idx=np.ascontiguousarray(np.tile(ident, (P // 16, 1))),
        wcat=np.ascontiguousarray(np.transpose(
            np.einsum("oc,kcd->kod", np.asarray(inputs["cpe_lin_w"], f32),
                      np.asarray(inputs["cpe_w"], f32)), (0, 2, 1))).astype(bf),
        qkv_wT=w8(inputs["qkv_w"]),
        proj_wT=w8(inputs["proj_w"]),
        fc1_wT=w8(inputs["fc1_w"]),
        fc2_wT=w8(inputs["fc2_w"]),
        cpe_b=pp(np.asarray(inputs["cpe_lin_w"], f32)
                 @ np.asarray(inputs["cpe_b"], f32)
                 + np.asarray(inputs["cpe_lin_b"], f32)),
        cpe_ln_g=pp(inputs["cpe_ln_g"]), cpe_ln_b=pp(inputs["cpe_ln_b"]),
        ln1_g=pp(inputs["ln1_g"]), ln1_b=pp(inputs["ln1_b"]),
        ln2_g=pp(inputs["ln2_g"]), ln2_b=pp(inputs["ln2_b"]),
        q_b=pp(qkv_b[:C] * SCALE), k_b=pp(qkv_b[C:2 * C]),
        v_b_rep=np.ascontiguousarray(np.broadcast_to(qkv_b[2 * C:], (P, C))),
        proj_b=pp(inputs["proj_b"]),
        fc1_b=pp(inputs["fc1_b"]), fc2_b=pp(inputs["fc2_b"]),
    )


def prep_core(inputs, core, nchunk):
    f32 = np.float32
    bf = ml_dtypes.bfloat16
    order = np.asarray(inputs["order"])
    feat = np.asarray(inputs["feat"], f32)
    nbr = np.asarray(inputs["neighbor_idx"])
    rows = order[core * R:(core + 1) * R]

    featT_own = np.ascontiguousarray(
        feat[rows].T.reshape(CC, P, R).transpose(1, 0, 2)).astype(bf)

    nb = nbr[rows]
    srcs, dsts = [], []
    for k, nk in nchunk.items():
        v = np.nonzero(nb[:, k] >= 0)[0]
        src = np.full(nk * P, NFULL, np.int64)
        dst = np.full(nk * P, TRASH, np.int64)
        src[:len(v)] = nb[v, k]
        dst[:len(v)] = v
        srcs.append(src)
        dsts.append(dst)
    src_all = np.concatenate(srcs)
    dst_all = np.concatenate(dsts)

    featp = np.vstack([feat, np.zeros((1, C), f32)])
    u = featp[src_all]                                    # [NCPAD, C]
    u_fm = np.ascontiguousarray(
        u.T.reshape(CC, P, -1).transpose(1, 0, 2)).astype(bf)

    sca = dst_all.astype(np.int16).reshape(-1, 16).T      # [16, NCPAD//16]
    sca_idx = np.ascontiguousarray(np.tile(sca, (P // 16, 1)))
    return dict(featT_own=featT_own, u_nc=u_fm, sca_idx=sca_idx), rows


def unshard_out(res_outT):
    return np.ascontiguousarray(
        np.asarray(res_outT).transpose(1, 0, 2).reshape(C, R).T)


# ======================= public entry point =======================

_CACHED = {}


def get_program(inputs):
    """Build (or fetch) the program for these inputs' sparsity pattern."""
    nchunk = compute_nchunk(inputs["neighbor_idx"], inputs["order"])
    key = tuple(sorted(nchunk.items()))
    if key not in _CACHED:
        _CACHED[key] = build_program(nchunk)
    return _CACHED[key], nchunk


def kernel(**inputs) -> np.ndarray:
    """Full-input, full-output entry. Shards across 8 NeuronCores by
    serialized patches (2 per core), runs the Bass kernel, scatters the
    per-core outputs back to original point order."""
    from concourse.bass_utils import run_bass_kernel_spmd

    inputs = {k: np.asarray(v) for k, v in inputs.items()}
    nc, nchunk = get_program(inputs)
    sh = prep_shared(inputs)
    in_maps, rows_l = [], []
    for c in range(8):
        ci, rows = prep_core(inputs, c, nchunk)
        in_maps.append({**sh, **ci})
        rows_l.append(rows)

    res = None
    last_err = None
    for attempt in range(3):
        try:
            res = run_bass_kernel_spmd(nc, in_maps, core_ids=list(range(8))).results
            break
        except Exception as e:   # transient NRT/axon hiccups: retry
            last_err = e
            import time as _t
            _t.sleep(2.0)
    if res is None:
        raise last_err

    out = np.zeros((NFULL, C), np.float32)
    for c in range(8):
        out[rows_l[c]] = unshard_out(res[c]["outT"])
    return out

